# revision 10
# baseline (speedup 1.0000x reference)
"""Mixtral MoE layer (8 experts, top-2, H=2048, I=7168, T=8192) on 8 trn2 NeuronCores.

Expert-parallel: core e owns expert e's FFN weights. The router (gate matmul +
softmax + top-2 + renormalize) runs on host CPU with the exact op sequence of
the reference; tokens are gathered per expert on host (the "all-to-all
dispatch"), each core runs the heavy FFN over its expert's tokens in bf16 with
fp32 PSUM accumulation, and the host scatter-adds the weighted expert outputs
back (the "all-to-all combine").

Device-side layout avoids all on-chip transposes:
  phase A:  Gt[i, c] = silu(W1t.T @ Xt) * (W3t.T @ Xt)   (inter on partitions)
  phase B:  Yt[h, c] += W2t.T @ Gt                        (hidden on partitions)
with Xt = X.T, W1t = w1.T, etc., all pre-tiled on host for contiguous DMA runs.
"""

import math

import numpy as np
import ml_dtypes

import concourse.bass as bass
import concourse.mybir as mybir
import concourse.tile as tile
from concourse.bass_utils import run_bass_kernel_spmd

H = 2048          # hidden dim
I = 7168          # intermediate dim
E = 8             # experts = cores
TOPK = 2
HJ = H // 128     # 16 hidden chunks of 128
IGW = 256         # phase-A inter group width
IG = I // IGW     # 28 groups
IK = I // 128     # 56 inter chunks of 128 for phase B
IKG = 8           # phase-B psum accumulation group size (56 = 7*8)
TBMAX = 512       # token block (matmul free dim)

BF16 = mybir.dt.bfloat16
F32 = mybir.dt.float32

last_exec_time_ns = None  # set when BASS_MOE_TRACE=1
last_results = None


def _install_axon_hooks_shim():
    """This image lacks antenv.axon_hooks (needed by run_bass_kernel_spmd
    trace=True). Provide it, with the NTFF profile hook driven via ctypes
    into the injected axon .so (mirrors trn_agent_boot._ntff_profile_via_ctypes)."""
    import sys

    try:
        import antenv.axon_hooks  # noqa: F401

        return
    except ImportError:
        pass
    import contextlib
    import ctypes
    import types

    hook = None
    so_path = "/opt/axon/libaxon_pjrt.so"
    try:
        lib = ctypes.CDLL(so_path)
        if hasattr(lib, "axon_start_nrt_profile"):
            lib.axon_start_nrt_profile.argtypes = [
                ctypes.POINTER(ctypes.c_int64),
                ctypes.c_size_t,
            ]
            lib.axon_start_nrt_profile.restype = ctypes.c_int64
            lib.axon_stop_nrt_profile.argtypes = [ctypes.c_char_p]
            lib.axon_stop_nrt_profile.restype = ctypes.c_int64

            @contextlib.contextmanager
            def _hook(output_dir, device_ids):
                import jax

                jax.devices()
                if device_ids:
                    ids = (ctypes.c_int64 * len(device_ids))(*device_ids)
                    rc = lib.axon_start_nrt_profile(ids, len(device_ids))
                else:
                    rc = lib.axon_start_nrt_profile(None, 0)
                if rc != 0:
                    raise RuntimeError(f"axon_start_nrt_profile rc={rc}")
                try:
                    yield
                finally:
                    n = lib.axon_stop_nrt_profile(str(output_dir).encode())
                    print(f"ntff profile: {n} file(s) -> {output_dir}", flush=True)

            hook = _hook
    except OSError:
        pass

    mod = types.ModuleType("antenv.axon_hooks")
    mod._hook = hook
    mod.get_axon_ntff_profile_hook = lambda: mod._hook
    mod.set_axon_ntff_profile_hook = lambda h: setattr(mod, "_hook", h)
    sys.modules["antenv.axon_hooks"] = mod


_install_axon_hooks_shim()


def legalize_single_wait(nc):
    """This walrus rejects >1 sem wait per instruction: hoist extras onto
    preceding NoOps on the same engine (per-engine program order preserved)."""
    n_split = 0
    for fn in nc.m.functions:
        for blk in fn.blocks:
            new = []
            for inst in blk.instructions:
                si = inst.sync_info
                if si is not None and si.on_wait and len(si.on_wait) > 1:
                    waits = list(si.on_wait)
                    for i, w in enumerate(waits[:-1]):
                        nop = mybir.InstNoOp(name=f"{inst.name}-w{i}", ins=[], outs=[])
                        nop.engine = inst.engine
                        nop.sync_info = mybir.SyncInfo(on_wait=[w], on_update=[])
                        new.append(nop)
                        n_split += 1
                    inst.sync_info = mybir.SyncInfo(
                        on_wait=[waits[-1]], on_update=list(si.on_update)
                    )
                new.append(inst)
            blk.instructions[:] = new
    return n_split


_programs = {}


def _build_program(C):
    """One SPMD program: FFN for C (padded) tokens of one expert."""
    if C in _programs:
        return _programs[C]

    nc = bass.Bass("TRN2", target_bir_lowering=False, debug=False, num_devices=E)
    xt = nc.declare_dram_parameter("xt", [HJ, 128, C], BF16, isOutput=False)
    w1 = nc.declare_dram_parameter("w1", [IG, HJ, 128, IGW], BF16, isOutput=False)
    w3 = nc.declare_dram_parameter("w3", [IG, HJ, 128, IGW], BF16, isOutput=False)
    w2 = nc.declare_dram_parameter("w2", [IK, 128, H], BF16, isOutput=False)
    yt = nc.declare_dram_parameter("yt", [HJ, 128, C], F32, isOutput=True)

    assert C % 16 == 0
    n_blocks = math.ceil(C / TBMAX)
    # near-equal block sizes (all 16-aligned, >=384 for C>=1920) keep every
    # matmul stream-bound; a ragged small tail block would be LDWEIGHTS-bound
    base = (C // n_blocks) // 16 * 16
    rem = (C - base * n_blocks) // 16
    tbs = [base + (16 if i < rem else 0) for i in range(n_blocks)]
    offs = [sum(tbs[:i]) for i in range(n_blocks)]
    assert sum(tbs) == C and all(t <= TBMAX for t in tbs)

    with tile.TileContext(nc) as tc:
        with (
            tc.tile_pool(name="xp", bufs=2) as xp,
            tc.tile_pool(name="w1p", bufs=2) as w1p,
            tc.tile_pool(name="w3p", bufs=2) as w3p,
            tc.tile_pool(name="w2p", bufs=10) as w2p,
            tc.tile_pool(name="gtp", bufs=IK) as gtp,
            tc.tile_pool(name="sip", bufs=3) as sip,
            tc.tile_pool(name="otp", bufs=HJ) as otp,
            tc.tile_pool(name="pga", bufs=2, space="PSUM") as pga,
            tc.tile_pool(name="pob", bufs=4, space="PSUM") as pob,
        ):
            for cb in range(n_blocks):
                c0 = offs[cb]
                tb = tbs[cb]

                xsb = xp.tile([128, HJ, tb], BF16, tag="xsb")
                nc.sync.dma_start(
                    out=xsb[:, :, :],
                    in_=xt[:, :, c0 : c0 + tb].rearrange("j p c -> p j c"),
                )

                # ---- phase A: Gt[i, c] for all 7168 inter rows ----
                gts = []
                for ig in range(IG):
                    w1sb = w1p.tile([128, HJ, IGW], BF16, tag="w1sb")
                    nc.sync.dma_start(
                        out=w1sb[:, :, :], in_=w1[ig].rearrange("j p i -> p j i")
                    )
                    w3sb = w3p.tile([128, HJ, IGW], BF16, tag="w3sb")
                    nc.scalar.dma_start(
                        out=w3sb[:, :, :], in_=w3[ig].rearrange("j p i -> p j i")
                    )
                    for m in range(IGW // 128):
                        pg1 = pga.tile([128, tb], F32, tag="pg1")
                        pg3 = pga.tile([128, tb], F32, tag="pg3")
                        ms = slice(m * 128, (m + 1) * 128)
                        for k in range(HJ):
                            nc.tensor.matmul(
                                pg1[:, :],
                                lhsT=w1sb[:, k, ms],
                                rhs=xsb[:, k, :],
                                start=(k == 0),
                                stop=(k == HJ - 1),
                            )
                        for k in range(HJ):
                            nc.tensor.matmul(
                                pg3[:, :],
                                lhsT=w3sb[:, k, ms],
                                rhs=xsb[:, k, :],
                                start=(k == 0),
                                stop=(k == HJ - 1),
                            )
                        ssb = sip.tile([128, tb], F32, tag="ssb")
                        nc.scalar.activation(
                            ssb[:, :], pg1[:, :], mybir.ActivationFunctionType.Silu
                        )
                        gt = gtp.tile([128, tb], BF16, tag="gt")
                        nc.vector.tensor_mul(gt[:, :], pg3[:, :], ssb[:, :])
                        gts.append(gt)

                # ---- phase B: Yt[h, c] = sum_i W2t[i, h] * Gt[i, c] ----
                outs = []
                for g in range(IK // IKG):
                    w2sbs = []
                    for u in range(IKG):
                        w2sb = w2p.tile([128, H], BF16, tag="w2sb")
                        nc.scalar.dma_start(out=w2sb[:, :], in_=w2[g * IKG + u])
                        w2sbs.append(w2sb)
                    for h in range(HJ):
                        po = pob.tile([128, tb], F32, tag="po")
                        hs = slice(h * 128, (h + 1) * 128)
                        for u in range(IKG):
                            nc.tensor.matmul(
                                po[:, :],
                                lhsT=w2sbs[u][:, hs],
                                rhs=gts[g * IKG + u][:, :],
                                start=(u == 0),
                                stop=(u == IKG - 1),
                            )
                        if g == 0:
                            ot = otp.tile([128, tb], F32, tag="ot")
                            nc.vector.tensor_copy(ot[:, :], po[:, :])
                            outs.append(ot)
                        else:
                            nc.vector.tensor_add(outs[h][:, :], outs[h][:, :], po[:, :])

                for h in range(HJ):
                    nc.sync.dma_start(
                        out=yt[h, :, c0 : c0 + tb], in_=outs[h][:, :]
                    )

    legalize_single_wait(nc)
    _programs[C] = nc
    return nc


def _routing(x, gate_weight):
    """Replicate the reference router bitwise-closely: jax on CPU, same ops."""
    import jax
    import jax.numpy as jnp

    cpu = jax.devices("cpu")[0]
    with jax.default_device(cpu):
        router_logits = jnp.asarray(x) @ jnp.asarray(gate_weight).T
        probs = jax.nn.softmax(router_logits.astype(jnp.float32), axis=-1)
        top_w, top_idx = jax.lax.top_k(probs, TOPK)
        top_w = top_w / jnp.sum(top_w, axis=-1, keepdims=True)
        top_w = top_w.astype(x.dtype)
        return np.asarray(top_w), np.asarray(top_idx)


def kernel(hidden_states, gate_weight, w1_weight, w3_weight, w2_weight):
    import os

    x = np.asarray(hidden_states, dtype=np.float32)
    T = x.shape[0]
    top_w, top_idx = _routing(x, np.asarray(gate_weight, dtype=np.float32))

    tok_ids = []
    tok_w = []
    for e in range(E):
        rows, cols = np.nonzero(top_idx == e)
        tok_ids.append(rows)
        tok_w.append(top_w[rows, cols].astype(np.float32))
    C = max(512, math.ceil(max(len(t) for t in tok_ids) / 16) * 16)

    bf16 = ml_dtypes.bfloat16
    in_maps = []
    for e in range(E):
        n_e = len(tok_ids[e])
        xg = np.zeros((C, H), dtype=bf16)
        xg[:n_e] = x[tok_ids[e]]
        xt = np.ascontiguousarray(xg.T).reshape(HJ, 128, C)

        w1t = np.ascontiguousarray(
            np.asarray(w1_weight[e], dtype=bf16).reshape(IG, IGW, HJ, 128)
            .transpose(0, 2, 3, 1)
        )
        w3t = np.ascontiguousarray(
            np.asarray(w3_weight[e], dtype=bf16).reshape(IG, IGW, HJ, 128)
            .transpose(0, 2, 3, 1)
        )
        w2t = np.ascontiguousarray(
            np.asarray(w2_weight[e], dtype=bf16).T
        ).reshape(IK, 128, H)
        in_maps.append({"xt": xt, "w1": w1t, "w3": w3t, "w2": w2t})

    nc = _build_program(C)
    trace = os.environ.get("BASS_MOE_TRACE", "") == "1"
    res = None
    if trace:
        import concourse.bass_utils as bu

        orig_upload = bu.upload_artifacts
        bu.upload_artifacts = lambda tmpdir: f"local://{tmpdir}"
        tdir = os.environ.get("BASS_MOE_TRACE_DIR") or None
        try:
            res = run_bass_kernel_spmd(
                nc, in_maps, list(range(E)), trace=True, tmpdir=tdir
            )
        except Exception as exc:
            print(f"trace path failed ({type(exc).__name__}: {exc}); rerunning untraced", flush=True)
            res = None
        finally:
            bu.upload_artifacts = orig_upload
    if res is None:
        res = run_bass_kernel_spmd(nc, in_maps, list(range(E)))
    global last_exec_time_ns, last_results
    last_exec_time_ns = res.exec_time_ns
    last_results = res

    out = np.zeros((T, H), dtype=np.float32)
    for e in range(E):
        n_e = len(tok_ids[e])
        yt = res.results[e]["yt"].reshape(H, C)
        out[tok_ids[e]] += tok_w[e][:, None] * yt[:, :n_e].T
    return out
